# revision 41
# baseline (speedup 1.0000x reference)
"""Trainium2 Bass kernel for nn_GunnarODE: neural CDE with hermite spline control.

Contract: kernel(**inputs) takes FULL unsharded inputs (ts, us, ys, W1, b1,
W2, b2, batch_size) and returns the FULL (B, L, Y) output. Internally shards
the batch across 8 NeuronCores (pure data parallel), runs a Bass/Tile kernel
per core, and reassembles.

Algorithm notes (derived from the reference):
  - x = concat([t, us]) with unit-spaced knots (ts is arange) => dt == 1.
  - Hermite backward-difference spline derivative at substep s_i = i/4 of
    interval k reduces to dXdt_i = alpha_i * slope_{k-1} + beta_i * slope_k
    with alpha_i = 1-4s+3s^2, beta_i = 4s-3s^2 (alpha+beta=1), i.e. a linear
    blend of (u_{k-1}, u_k, u_{k+1}); the time channel has dXdt == 1.
  - Per Euler substep: h = tanh(z@W1.T+b1); vf = tanh(h@W2.T+b2) viewed as
    (Y=16, C=9); z += 0.25 * einsum(vf, dXdt).
  - On device everything is kept transposed (feature on partitions, batch on
    the free dim). The 144 vf rows are split into 128 "ctrl" rows
    (r=(c-1)*16+y for channels c=1..8) and 16 "time" rows (y*9).
  - All matmuls are fp32: the ODE amplifies per-step rounding ~1e5x, so
    reduced-precision matmuls (fp32r/bf16) fail the accuracy budget.
"""
import sys
if '/opt/trn_rl_repo' not in sys.path:
    sys.path.insert(0, '/opt/trn_rl_repo')

import numpy as np

N_CORES = 8
L = 512
B_TOT = 4096
U = 8
Y = 16
H = 128
C = U + 1
NI = L - 1            # intervals
HSTEP = 0.25          # dt / SUBSTEPS with dt == 1
B_LOC = B_TOT // N_CORES  # 512

ALPHA = [1.0, 0.1875, -0.25, -0.3125]
BETA = [0.0, 0.8125, 1.25, 1.3125]

_BUILD_CACHE = {}


def _host_constants(W1, b1, W2, b2):
    """Precompute transposed/permuted constant matrices (host-side, free)."""
    rowmap = np.array([(r % 16) * 9 + (r // 16 + 1) for r in range(128)])
    cst = {}
    cst["W1T"] = np.ascontiguousarray(W1.T)                        # (16,128)
    cst["W2aT"] = np.ascontiguousarray(W2[rowmap, :].T)            # (128,128)
    cst["W2bT"] = np.ascontiguousarray(W2[np.arange(16) * 9, :].T)  # (128,16)
    cst["b1c"] = np.ascontiguousarray(b1[:, None])                 # (128,1)
    cst["b2c"] = np.ascontiguousarray(b2[rowmap][:, None])         # (128,1)
    cst["b2t"] = np.ascontiguousarray(b2[np.arange(16) * 9][:, None])  # (16,1)
    abc = np.zeros((4, 24, 128), dtype=np.float32)
    for i in range(4):
        for r in range(128):
            c = r // 16 + 1
            abc[i, 0 * 8 + c - 1, r] = -ALPHA[i]
            abc[i, 1 * 8 + c - 1, r] = ALPHA[i] - BETA[i]
            abc[i, 2 * 8 + c - 1, r] = BETA[i]
    cst["Abc"] = abc                                               # (4,24,128)
    # hpre-state update matrices: hpre += (h*W1*Sel^T) @ tmp + (h*W1) @ vft
    w1selt = np.zeros((128, 128), dtype=np.float32)  # [r, j] = h*W1[j, r%16]
    for r in range(128):
        w1selt[r, :] = HSTEP * W1[:, r % 16]
    cst["W1SelT"] = w1selt
    cst["W1hT"] = (HSTEP * W1.T)                                   # (16,128)
    # output reconstruction: z = pinv(W1) @ hpre  (W1 is 128x16, cond ~2)
    R = np.linalg.pinv(W1.astype(np.float64)).astype(np.float32)   # (16,128)
    cst["RT"] = np.ascontiguousarray(R.T)                          # (128,16)
    return {k: v.astype(np.float32) for k, v in cst.items()}


def _build(n_intervals=NI):
    """Build + compile the Bass module (cached per interval count)."""
    key = n_intervals
    if key in _BUILD_CACHE:
        return _BUILD_CACHE[key]

    import concourse.bass as bass
    import concourse.bacc as bacc
    import concourse.tile as tile
    from concourse import mybir

    F32 = mybir.dt.float32
    TANH = mybir.ActivationFunctionType.Tanh
    MULT = mybir.AluOpType.mult
    ADD = mybir.AluOpType.add

    nc = bacc.Bacc("TRN2", target_bir_lowering=False, debug=False,
                   num_devices=N_CORES)

    nsub = 4 * n_intervals
    d_dx = nc.dram_tensor("dx", (nsub, U, B_LOC), F32, kind="ExternalInput")
    d_ys0 = nc.dram_tensor("ys0T", (16, B_LOC), F32, kind="ExternalInput")
    d_W1T = nc.dram_tensor("W1T", (16, 128), F32, kind="ExternalInput")
    d_W2aT = nc.dram_tensor("W2aT", (128, 128), F32, kind="ExternalInput")
    d_W2bT = nc.dram_tensor("W2bT", (128, 16), F32, kind="ExternalInput")
    d_b1 = nc.dram_tensor("b1c", (128, 1), F32, kind="ExternalInput")
    d_b2c = nc.dram_tensor("b2c", (128, 1), F32, kind="ExternalInput")
    d_b2t = nc.dram_tensor("b2t", (16, 1), F32, kind="ExternalInput")
    d_W1SelT = nc.dram_tensor("W1SelT", (128, 128), F32, kind="ExternalInput")
    d_W1hT = nc.dram_tensor("W1hT", (16, 128), F32, kind="ExternalInput")
    d_out = nc.dram_tensor("out", (n_intervals, 128, B_LOC), F32, kind="ExternalOutput")

    with tile.TileContext(nc) as tc:
        with (
            tc.tile_pool(name="consts", bufs=1) as consts,
            tc.tile_pool(name="zpool", bufs=3) as zpool,
            tc.tile_pool(name="work", bufs=2) as work,
            tc.tile_pool(name="u3p", bufs=6) as u3p,
            tc.tile_pool(name="ps1", bufs=1, space="PSUM") as ps1,
            tc.tile_pool(name="ps2", bufs=2, space="PSUM") as ps2,
        ):
            W1T = consts.tile([16, 128], F32)
            W2aT = consts.tile([128, 128], F32)
            W2bT = consts.tile([128, 16], F32)
            b1c = consts.tile([128, 1], F32)
            b2c = consts.tile([128, 1], F32)
            b2t = consts.tile([16, 1], F32)
            W1SelT = consts.tile([128, 128], F32)
            W1hT = consts.tile([16, 128], F32)
            nc.sync.dma_start(W1T[:], d_W1T.ap())
            nc.sync.dma_start(W2aT[:], d_W2aT.ap())
            nc.sync.dma_start(W2bT[:], d_W2bT.ap())
            nc.sync.dma_start(b1c[:], d_b1.ap())
            nc.sync.dma_start(b2c[:], d_b2c.ap())
            nc.sync.dma_start(b2t[:], d_b2t.ap())
            nc.sync.dma_start(W1SelT[:], d_W1SelT.ap())
            nc.sync.dma_start(W1hT[:], d_W1hT.ap())

            z0 = zpool.tile([16, B_LOC], F32, tag="z")
            nc.sync.dma_start(z0[:], d_ys0.ap())

            # hpre is THE state: a persistent PSUM accumulator holding W1 @ z.
            # Each substep adds W1 @ dz via one K=128 + one K=16 matmul; z is
            # only reconstructed per interval for output via R = pinv(W1).
            hpre = ps1.tile([128, B_LOC], F32, tag="hpre")
            nc.tensor.matmul(hpre[:], W1T[:], z0[:], start=True, stop=False,
                             skip_group_check=True)

            HB = B_LOC // 2
            dxs = {}

            def load_dx(g):
                if g < nsub:
                    t = u3p.tile([128, B_LOC], F32, tag="dx", name=f"dx_{g}")
                    src = d_dx.ap()[g].unsqueeze(1).broadcast_to((U, 16, B_LOC))
                    nc.sync.dma_start(t[:], src)
                    dxs[g] = t

            for g in range(5):
                load_dx(g)
            for k in range(n_intervals):
                for i in range(4):
                    g = 4 * k + i
                    load_dx(g + 5)
                    dXb = dxs.pop(g)
                    th = work.tile([128, B_LOC], F32, tag="th")
                    # column-split pipeline: tanh_h half 0 -> MM2a half 0
                    # overlaps tanh_h half 1 -> MM2a half 1
                    nc.scalar.activation(th[:, :HB], hpre[:, :HB], TANH, bias=b1c[:])
                    nc.scalar.activation(th[:, HB:], hpre[:, HB:], TANH, bias=b1c[:])
                    vfc_h = [ps1.tile([128, HB], F32, tag=f"vfc{h}", name=f"vfc{h}_{k}_{i}")
                             for h in range(2)]
                    nc.tensor.matmul(vfc_h[0][:], W2aT[:], th[:, :HB],
                                     start=True, stop=True)
                    nc.tensor.matmul(vfc_h[1][:], W2aT[:], th[:, HB:],
                                     start=True, stop=True)
                    vft_ps = ps1.tile([16, B_LOC], F32, tag="vft")
                    nc.tensor.matmul(vft_ps[:], W2bT[:], th[:], start=True, stop=True)
                    vfc = work.tile([128, B_LOC], F32, tag="vfcs")
                    nc.scalar.activation(vfc[:, :HB], vfc_h[0][:], TANH, bias=b2c[:])
                    nc.scalar.activation(vfc[:, HB:], vfc_h[1][:], TANH, bias=b2c[:])
                    vft = work.tile([16, B_LOC], F32, tag="vfts")
                    nc.scalar.activation(vft[:], vft_ps[:], TANH, bias=b2t[:])
                    tmp = work.tile([128, B_LOC], F32, tag="tmp")
                    nc.vector.tensor_tensor(tmp[:, :HB], vfc[:, :HB], dXb[:, :HB],
                                            MULT)
                    # chain-critical state update, half-pipelined:
                    # hpre += (h*W1*Sel^T)@tmp + (h*W1)@vft
                    nc.tensor.matmul(hpre[:, :HB], W1SelT[:], tmp[:, :HB],
                                     start=False, stop=False, skip_group_check=True)
                    nc.vector.tensor_tensor(tmp[:, HB:], vfc[:, HB:], dXb[:, HB:],
                                            MULT)
                    nc.tensor.matmul(hpre[:, HB:], W1SelT[:], tmp[:, HB:],
                                     start=False, stop=False, skip_group_check=True)
                    nc.tensor.matmul(hpre[:, :HB], W1hT[:], vft[:, :HB],
                                     start=False, stop=False,
                                     skip_group_check=True)
                    nc.tensor.matmul(hpre[:, HB:], W1hT[:], vft[:, HB:],
                                     start=False, stop=False,
                                     skip_group_check=True)
                # per-interval output: snapshot hpre; host recovers
                # z_{k+1} = pinv(W1) @ hpre.
                hps = work.tile([128, B_LOC], F32, tag="hps")
                nc.vector.tensor_copy(hps[:], hpre[:])
                nc.sync.dma_start(d_out.ap()[k], hps[:])

    nc.compile()
    _BUILD_CACHE[key] = nc
    return nc


def _prep_core_inputs(us, ys, cst, core, n_intervals):
    b0 = core * B_LOC
    usT = np.ascontiguousarray(us[:, b0:b0 + B_LOC, :].transpose(0, 2, 1))
    sl = usT[1:] - usT[:-1]                          # (L-1, 8, B) slopes
    sm1 = np.concatenate([sl[:1], sl[:-1]], axis=0)  # backward-shifted
    sl = sl[:n_intervals]
    sm1 = sm1[:n_intervals]
    dx = np.stack([ALPHA[i] * sm1 + BETA[i] * sl for i in range(4)], axis=1)
    dx = np.ascontiguousarray(dx.reshape(4 * n_intervals, U, B_LOC),
                              dtype=np.float32)
    ys0T = np.ascontiguousarray(ys[0, b0:b0 + B_LOC, :].T).astype(np.float32)
    m = {"dx": dx, "ys0T": ys0T}
    m.update({k: v for k, v in cst.items() if k not in ("Abc", "RT")})
    return m


def kernel(ts, us, ys, W1, b1, W2, b2, batch_size=None, n_intervals=NI):
    from concourse.bass_utils import run_bass_kernel_spmd

    us = np.asarray(us, dtype=np.float32)
    ys = np.asarray(ys, dtype=np.float32)
    W1 = np.asarray(W1, np.float32)
    cst = _host_constants(W1, np.asarray(b1, np.float32),
                          np.asarray(W2, np.float32), np.asarray(b2, np.float32))
    nc = _build(n_intervals)
    in_maps = [_prep_core_inputs(us, ys, cst, c, n_intervals) for c in range(N_CORES)]
    res = run_bass_kernel_spmd(nc, in_maps, core_ids=list(range(N_CORES)))
    # output reconstruction: z = pinv(W1) @ hpre  (W1 is 128x16, cond ~2)
    R = np.linalg.pinv(W1.astype(np.float64)).astype(np.float32)   # (16,128)
    out = np.empty((B_TOT, n_intervals + 1, Y), dtype=np.float32)
    out[:, 0, :] = ys[0]
    for c in range(N_CORES):
        b0 = c * B_LOC
        hout = res.results[c]["out"]                 # (NI, 128, B_LOC)
        z = np.tensordot(R, hout, axes=(1, 1))       # (16, NI, B_LOC)
        out[b0:b0 + B_LOC, 1:, :] = z.transpose(2, 1, 0)
    kernel._last_results = res
    return out



# revision 43
# speedup vs baseline: 1.0005x; 1.0005x over previous
"""Trainium2 Bass kernel for nn_GunnarODE: neural CDE with hermite spline control.

Contract: kernel(**inputs) takes FULL unsharded inputs (ts, us, ys, W1, b1,
W2, b2, batch_size) and returns the FULL (B, L, Y) output. Internally shards
the batch across 8 NeuronCores (pure data parallel), runs a Bass/Tile kernel
per core, and reassembles.

Algorithm notes (derived from the reference):
  - x = concat([t, us]) with unit-spaced knots (ts is arange) => dt == 1.
  - Hermite backward-difference spline derivative at substep s_i = i/4 of
    interval k reduces to dXdt_i = alpha_i * slope_{k-1} + beta_i * slope_k
    with alpha_i = 1-4s+3s^2, beta_i = 4s-3s^2; the time channel has
    dXdt == 1.
  - Per Euler substep: h = tanh(z@W1.T+b1); vf = tanh(h@W2.T+b2) viewed as
    (Y=16, C=9); z += 0.25 * einsum(vf, dXdt).
  - On device everything is kept transposed (feature on partitions, batch on
    the free dim). The 144 vf rows are split into 128 "ctrl" rows
    (r=(c-1)*16+y for channels c=1..8) and 16 "time" rows (y*9).
  - All matmuls are fp32: the ODE amplifies per-step rounding ~1e5x, so
    reduced-precision matmuls (fp32r/bf16) fail the accuracy budget.

Performance structure: only 3 fp32 matmul passes over the batch per substep:
  1. yva = W2a @ th   (128 ctrl pre-activations, column halves)
  2. yvb = W2b @ th   (16 time pre-activations, column halves)
  3. hpre += (h*W1*Sel^T) @ tmp  (state update, column halves)
The spline derivative dXdt is precomputed on the host for every (interval,
substep) and streamed in via a partition-broadcast DMA (8 channels -> 128
rows).  The time-channel state contribution is folded into tmp rows 0..15
with an in-place DVE add (those rows carry weight HSTEP*W1[:,y] in the
W1Sel matmul, so adding vft there adds exactly HSTEP*W1@vft to hpre).  Per
interval the hpre snapshot is DMA'd out and the z = pinv(W1) @ hpre
projection runs on the host.
"""
import sys
if '/opt/trn_rl_repo' not in sys.path:
    sys.path.insert(0, '/opt/trn_rl_repo')

import numpy as np

N_CORES = 8
L = 512
B_TOT = 4096
U = 8
Y = 16
H = 128
C = U + 1
NI = L - 1            # intervals
HSTEP = 0.25          # dt / SUBSTEPS with dt == 1
B_LOC = B_TOT // N_CORES  # 512

ALPHA = [1.0, 0.1875, -0.25, -0.3125]
BETA = [0.0, 0.8125, 1.25, 1.3125]

_BUILD_CACHE = {}


def _host_constants(W1, b1, W2, b2):
    """Precompute transposed/permuted constant matrices (host-side, free)."""
    rowmap = np.array([(r % 16) * 9 + (r // 16 + 1) for r in range(128)])
    cst = {}
    cst["W1T"] = np.ascontiguousarray(W1.T)                        # (16,128)
    cst["W2aT"] = np.ascontiguousarray(W2[rowmap, :].T)            # (128,128)
    cst["W2bT"] = np.ascontiguousarray(W2[np.arange(16) * 9, :].T)  # (128,16)
    cst["b1c"] = np.ascontiguousarray(b1[:, None])                 # (128,1)
    cst["b2c"] = np.ascontiguousarray(b2[rowmap][:, None])         # (128,1)
    cst["b2t"] = np.ascontiguousarray(b2[np.arange(16) * 9][:, None])  # (16,1)
    # state update matrix: hpre += (h*W1*Sel^T) @ tmp, [r, j] = h*W1[j, r%16]
    w1selt = np.zeros((128, 128), dtype=np.float32)
    for r in range(128):
        w1selt[r, :] = HSTEP * W1[:, r % 16]
    cst["W1SelT"] = w1selt
    return {k: v.astype(np.float32) for k, v in cst.items()}


def _build(n_intervals=NI):
    """Build + compile the Bass module (cached per interval count)."""
    key = n_intervals
    if key in _BUILD_CACHE:
        return _BUILD_CACHE[key]

    import concourse.bass as bass
    import concourse.bacc as bacc
    import concourse.tile as tile
    from concourse import mybir

    F32 = mybir.dt.float32
    TANH = mybir.ActivationFunctionType.Tanh
    MULT = mybir.AluOpType.mult
    ADD = mybir.AluOpType.add

    nsub = 4 * n_intervals

    nc = bacc.Bacc("TRN2", target_bir_lowering=False, debug=False,
                   num_devices=N_CORES)

    d_dx = nc.dram_tensor("dx", (nsub, U, B_LOC), F32, kind="ExternalInput")
    d_ys0 = nc.dram_tensor("ys0T", (16, B_LOC), F32, kind="ExternalInput")
    d_W1T = nc.dram_tensor("W1T", (16, 128), F32, kind="ExternalInput")
    d_W2aT = nc.dram_tensor("W2aT", (128, 128), F32, kind="ExternalInput")
    d_W2bT = nc.dram_tensor("W2bT", (128, 16), F32, kind="ExternalInput")
    d_b1 = nc.dram_tensor("b1c", (128, 1), F32, kind="ExternalInput")
    d_b2c = nc.dram_tensor("b2c", (128, 1), F32, kind="ExternalInput")
    d_b2t = nc.dram_tensor("b2t", (16, 1), F32, kind="ExternalInput")
    d_W1SelT = nc.dram_tensor("W1SelT", (128, 128), F32, kind="ExternalInput")
    d_hout = nc.dram_tensor("hout", (n_intervals, 128, B_LOC), F32,
                            kind="ExternalOutput")

    with tile.TileContext(nc) as tc:
        with (
            tc.tile_pool(name="consts", bufs=1) as consts,
            tc.tile_pool(name="work", bufs=2) as work,
            tc.tile_pool(name="dxp", bufs=6) as dxp,
            tc.tile_pool(name="hsp", bufs=2) as hsp,
            tc.tile_pool(name="ps1", bufs=1, space="PSUM") as ps1,
            tc.tile_pool(name="ps2", bufs=2, space="PSUM") as ps2,
        ):
            W1T = consts.tile([16, 128], F32)
            W2aT = consts.tile([128, 128], F32)
            W2bT = consts.tile([128, 16], F32)
            b1c = consts.tile([128, 1], F32)
            b2c = consts.tile([128, 1], F32)
            b2t = consts.tile([16, 1], F32)
            W1SelT = consts.tile([128, 128], F32)
            nc.sync.dma_start(W1T[:], d_W1T.ap())
            nc.sync.dma_start(W2aT[:], d_W2aT.ap())
            nc.sync.dma_start(W2bT[:], d_W2bT.ap())
            nc.sync.dma_start(b1c[:], d_b1.ap())
            nc.sync.dma_start(b2c[:], d_b2c.ap())
            nc.sync.dma_start(b2t[:], d_b2t.ap())
            nc.sync.dma_start(W1SelT[:], d_W1SelT.ap())

            z0 = consts.tile([16, B_LOC], F32)
            nc.sync.dma_start(z0[:], d_ys0.ap())

            # hpre is THE state: a persistent PSUM accumulator holding W1 @ z.
            hpre = ps1.tile([128, B_LOC], F32, tag="hpre")
            nc.tensor.matmul(hpre[:], W1T[:], z0[:], start=True, stop=False,
                             skip_group_check=True)

            HB = B_LOC // 2
            COLS = [(0, HB), (HB, B_LOC)]
            dxs = {}

            def load_dx(g):
                if g < nsub:
                    t = dxp.tile([128, B_LOC], F32, tag="dx", name=f"dx_{g}")
                    src = d_dx.ap()[g].unsqueeze(1).broadcast_to((U, 16, B_LOC))
                    nc.sync.dma_start(t[:], src)
                    dxs[g] = t

            for g in range(5):
                load_dx(g)

            for k in range(n_intervals):
                for i in range(4):
                    g = 4 * k + i
                    load_dx(g + 5)
                    dxt = dxs.pop(g)
                    th = work.tile([128, B_LOC], F32, tag="th")
                    yva = [ps2.tile([128, HB], F32, tag=f"yva{h}",
                                    name=f"yva{h}_{g}") for h in range(2)]
                    yvb = ps2.tile([16, B_LOC], F32, tag="yvb", name=f"yvb_{g}")
                    vfc = work.tile([128, B_LOC], F32, tag="vfc")
                    vft = work.tile([16, B_LOC], F32, tag="vft")
                    tmp = work.tile([128, B_LOC], F32, tag="tmp")

                    (c0, c1) = COLS[0]
                    nc.scalar.activation(th[:, c0:c1], hpre[:, c0:c1],
                                         TANH, bias=b1c[:])
                    nc.tensor.matmul(yva[0][:], W2aT[:], th[:, c0:c1],
                                     start=True, stop=True)
                    (d0, d1) = COLS[1]
                    nc.scalar.activation(th[:, d0:d1], hpre[:, d0:d1],
                                         TANH, bias=b1c[:])
                    nc.tensor.matmul(yvb[:, c0:c1], W2bT[:], th[:, c0:c1],
                                     start=True, stop=True)
                    nc.scalar.activation(vfc[:, c0:c1], yva[0][:], TANH,
                                         bias=b2c[:])
                    nc.scalar.activation(vft[:, c0:c1], yvb[:, c0:c1],
                                         TANH, bias=b2t[:])
                    nc.vector.tensor_tensor(tmp[:, c0:c1], vfc[:, c0:c1],
                                            dxt[:, c0:c1], MULT)
                    # tmp rows 0..15 (channel 1) also carry the time channel:
                    # + vft adds HSTEP*W1@vft to hpre below.
                    nc.vector.tensor_tensor(tmp[:16, c0:c1], tmp[:16, c0:c1],
                                            vft[:, c0:c1], ADD)
                    nc.tensor.matmul(yva[1][:], W2aT[:], th[:, d0:d1],
                                     start=True, stop=True)
                    nc.tensor.matmul(yvb[:, d0:d1], W2bT[:], th[:, d0:d1],
                                     start=True, stop=True)
                    nc.scalar.activation(vfc[:, d0:d1], yva[1][:], TANH,
                                         bias=b2c[:])
                    nc.scalar.activation(vft[:, d0:d1], yvb[:, d0:d1],
                                         TANH, bias=b2t[:])
                    nc.vector.tensor_tensor(tmp[:, d0:d1], vfc[:, d0:d1],
                                            dxt[:, d0:d1], MULT)
                    nc.vector.tensor_tensor(tmp[:16, d0:d1], tmp[:16, d0:d1],
                                            vft[:, d0:d1], ADD)
                    nc.tensor.matmul(hpre[:, c0:c1], W1SelT[:],
                                     tmp[:, c0:c1], start=False,
                                     stop=False, skip_group_check=True)
                    nc.tensor.matmul(hpre[:, d0:d1], W1SelT[:],
                                     tmp[:, d0:d1], start=False,
                                     stop=False, skip_group_check=True)
                # per-interval output: snapshot hpre; host recovers
                # z_{k+1} = pinv(W1) @ hpre.
                hps = hsp.tile([128, B_LOC], F32, tag="hps")
                nc.vector.tensor_copy(hps[:], hpre[:])
                nc.sync.dma_start(d_hout.ap()[k], hps[:])

    nc.compile()
    _BUILD_CACHE[key] = nc
    return nc


def _prep_core_inputs(us, ys, cst, core, n_intervals):
    b0 = core * B_LOC
    usT = np.ascontiguousarray(us[:, b0:b0 + B_LOC, :].transpose(0, 2, 1))
    sl = usT[1:] - usT[:-1]                          # (L-1, 8, B) slopes
    sm1 = np.concatenate([sl[:1], sl[:-1]], axis=0)  # backward-shifted
    sl = sl[:n_intervals]
    sm1 = sm1[:n_intervals]
    dx = np.stack([ALPHA[i] * sm1 + BETA[i] * sl for i in range(4)], axis=1)
    dx = np.ascontiguousarray(dx.reshape(4 * n_intervals, U, B_LOC),
                              dtype=np.float32)
    ys0T = np.ascontiguousarray(ys[0, b0:b0 + B_LOC, :].T).astype(np.float32)
    m = {"dx": dx, "ys0T": ys0T}
    m.update(cst)
    return m


def kernel(ts, us, ys, W1, b1, W2, b2, batch_size=None, n_intervals=NI):
    from concourse.bass_utils import run_bass_kernel_spmd

    us = np.asarray(us, dtype=np.float32)
    ys = np.asarray(ys, dtype=np.float32)
    W1 = np.asarray(W1, np.float32)
    cst = _host_constants(W1, np.asarray(b1, np.float32),
                          np.asarray(W2, np.float32), np.asarray(b2, np.float32))
    nc = _build(n_intervals)
    in_maps = [_prep_core_inputs(us, ys, cst, c, n_intervals) for c in range(N_CORES)]
    res = run_bass_kernel_spmd(nc, in_maps, core_ids=list(range(N_CORES)))
    # output reconstruction: z = pinv(W1) @ hpre  (W1 is 128x16, cond ~2)
    R = np.linalg.pinv(W1.astype(np.float64)).astype(np.float32)   # (16,128)
    out = np.empty((B_TOT, n_intervals + 1, Y), dtype=np.float32)
    out[:, 0, :] = ys[0]
    for c in range(N_CORES):
        b0 = c * B_LOC
        hout = res.results[c]["hout"]                # (NI, 128, B_LOC)
        z = np.tensordot(R, hout, axes=(1, 1))       # (16, NI, B_LOC)
        out[b0:b0 + B_LOC, 1:, :] = z.transpose(2, 1, 0)
    kernel._last_results = res
    return out
